# revision 21
# baseline (speedup 1.0000x reference)
"""Trainium2 Bass kernel for BottleneckAttention (patch attention).

q patches [160, 5120] from z1_hat (non-overlapping 10x4 unfold),
kv patches [5551, 5120] from z2 (overlapping unfold, Hk=91 x Wk=61),
scores = q @ kv.T / 5120, softmax over kv patches, out = attn @ kv,
folded back to [1, 128, 100, 64].

Sharding: 12 kv h-rows (768 flat positions) per core; every core computes
all 160 q columns; host combines with an all-gather softmax using the
centered form f = e - 1 (the exact colsum term is added in fp64 on host).

Per-core kernel (v14): every matmul is an fp8e4 DoubleRow matmul -- two
128-contraction k-tiles per instruction at ~78 ns (vs ~68 ns for one),
which beats the previous Winograd scheme outright and removes all the
vector-engine score combining.
  phase 1 computes scores TRANSPOSED [pos, q]: per 128-position chunk,
    taps pair over (i, i+1): the stationary pair is an overlapping
    [128, 2, 128] window AP (dim-1 stride 64) on the flat z slab; the
    moving pair is the host-interleaved q tap pair [128, 2, 160].
    Order: ip0 across all 6 chunks (only q piece 0 needed, so compute
    starts while the rest of q is still in flight), then ip1..4 for
    chunks 0/1 (their f feeds phase 2 early), then chunks 2..5.
    ScalarE applies exp straight from PSUM; VectorE forms f = (e-1)*mask
    in fp8.
  phase 2 computes out TRANSPOSED [(c,i,j), q]: per tap, 3 DoubleRow
    over position-chunk pairs: stationary = adjacent [128, 2, 128] slabs
    of the host-built zt (shifted z2T copies), moving = f chunk pairs.
    Batch 0 (taps 0-7) runs m-major so the phase-1 exp/f tail hides
    behind its m=0/m=1 passes. PSUM->SBUF copies alternate
    ScalarE/VectorE; fp16 output.
  denominator = ones-vector matmul chain after tap 28 in bank 6, shipped
    immediately on the Activation queue.
Input DMAs ride both HW-DGE queues (Activation: q0/q2/q4 + zt halves;
SP/Sync: zw, q1, q3, mask) in consumption order -- DMA completion has a
~2 us notify latency, so phase 1's start is input-bound; the 27 warmup
matmuls keep the PE p-state ramped until q piece 0 lands. Outputs ride
the SP queue in 8-tap groups as copies complete, with the last group
split off on the Activation queue.
"""

import sys

sys.path.insert(0, "/opt/trn_rl_repo")

import numpy as np
import ml_dtypes

import concourse.bass as bass
import concourse.mybir as mybir
import bass_rust

DT = mybir.dt
AF = mybir.ActivationFunctionType
ALU = mybir.AluOpType
PM = mybir.MatmulPerfMode

# problem geometry (hardcoded from the reference module)
KC, KH, KW = 128, 10, 4
H, W = 100, 64
NH, NW = H // KH, W // KW          # 10, 16
PQ = NH * NW                       # 160 q patches
D = KC * KH * KW                   # 5120
HK, WK = H - KH + 1, W - KW + 1    # 91, 61
NCORES = 8
HPC = 12                           # kv h-rows per core
NPOS = 24 * W                      # 1536 slab positions per core
NOWN = HPC * W                     # 768 owned positions per core
NB = NOWN // 128                   # 6 score/f chunks of 128 positions
NIP = KH // 2                      # 5 i-pairs
NIJ = KH * KW                      # 40 (i,j) output taps
NA = 10                            # zt 'a' slabs
DELTAS = (0, 1, 2, 3, 64, 65, 66, 67)
SCALE = 1.0 / D

F8 = ml_dtypes.float8_e4m3

_CACHE = {}

# phase-2 tap -> psum bank (taps 0,1 use the banks phase 1 never touches)
PERM = (6, 7, 0, 1, 2, 3, 4, 5)


def _build_nc():
    nc = bass.Bass()
    zw_d = nc.declare_dram_parameter("zw", [KC, NPOS], DT.float8e4, isOutput=False)
    q_d = nc.declare_dram_parameter(
        "qp", [KC, NIP, KW * 2 * PQ], DT.float8e4, isOutput=False
    )
    zt_d = nc.declare_dram_parameter("zt", [128, 8, NA, KC], DT.float8e4, isOutput=False)
    mk_d = nc.declare_dram_parameter("msk", [128, NB], DT.float32, isOutput=False)
    out_d = nc.declare_dram_parameter("out", [KC, NIJ, PQ], DT.float16, isOutput=True)
    den_d = nc.declare_dram_parameter("den", [1, PQ], DT.float32, isOutput=True)

    from contextlib import ExitStack

    ctx = ExitStack()
    with ctx:
        zw_sb = ctx.enter_context(nc.sbuf_tensor([KC, NPOS], DT.float8e4))
        q_sb = ctx.enter_context(nc.sbuf_tensor([KC, NIP, KW * 2 * PQ], DT.float8e4))
        zt_sb = ctx.enter_context(nc.sbuf_tensor([128, 8, NA, KC], DT.float8e4))
        mk_sb = ctx.enter_context(nc.sbuf_tensor([128, NB], DT.float32))
        e_sb = ctx.enter_context(nc.sbuf_tensor([128, NB, PQ], DT.float32))
        f_sb = ctx.enter_context(nc.sbuf_tensor([128, NB, PQ], DT.float8e4))
        o_sb = ctx.enter_context(nc.sbuf_tensor([128, NIJ, PQ], DT.float16))
        den_sb = ctx.enter_context(nc.sbuf_tensor([1, PQ], DT.float32))
        ones_sb = ctx.enter_context(nc.sbuf_tensor([128, 1], DT.float8e4))
        wz = ctx.enter_context(nc.sbuf_tensor([128, 128], DT.float8e4))

        ps = [
            ctx.enter_context(nc.psum_tensor(f"ps{i}", [128, 512], DT.float32))
            for i in range(8)
        ]

        s_wz = ctx.enter_context(nc.semaphore("s_wz"))
        s_izw = [ctx.enter_context(nc.semaphore(f"s_izw{i}")) for i in range(2)]
        s_qs = [ctx.enter_context(nc.semaphore(f"s_q{i}")) for i in range(NIP)]
        s_im = ctx.enter_context(nc.semaphore("s_im"))
        s_izt = [ctx.enter_context(nc.semaphore(f"s_izt{i}")) for i in range(2)]
        s_p = ctx.enter_context(nc.semaphore("s_p"))
        s_exp = ctx.enter_context(nc.semaphore("s_exp"))
        s_f = ctx.enter_context(nc.semaphore("s_f"))
        s_cpa = ctx.enter_context(nc.semaphore("s_cpa"))
        s_cpv = ctx.enter_context(nc.semaphore("s_cpv"))
        s_den = ctx.enter_context(nc.semaphore("s_den"))
        s_o = ctx.enter_context(nc.semaphore("s_o"))

        # s_p: phase-1 chunks 1..6; taps 0..28 -> 7..35; den -> 36;
        #      taps 29..39 -> 37..47
        DEN_AT = 28

        def sp_tap(g):
            return 7 + g if g <= DEN_AT else 8 + g

        def q_pair(ip, j):
            # contiguous tap pair [128, 2, 160] inside the flat q piece
            c = q_sb[:, ip, 2 * PQ * j : 2 * PQ * (j + 1)].copy()
            c.ap = bass_rust.VecI64Pair([[NIP * 2 * PQ * KW, 128], [PQ, 2], [1, PQ]])
            return c

        def zw_pair(off):
            # overlapping [128, 2, 128] stationary: windows at off, off+64
            c = zw_sb[:, off : off + 128].copy()
            c.ap = bass_rust.VecI64Pair([[NPOS, 128], [64, 2], [1, 128]])
            return c

        with nc.Block() as block:

            @block.sync
            def _(sync):
                # head inputs interleaved with the Activation queue: this
                # queue carries zw (split), q pieces 1/3 and the mask
                sync.dma_start(zw_sb[:], zw_d[:]).then_inc(s_izw[0], 16)
                sync.dma_start(q_sb[:, 1:2], q_d[:, 1:2]).then_inc(s_qs[1], 16)
                sync.dma_start(q_sb[:, 3:4], q_d[:, 3:4]).then_inc(s_qs[3], 16)
                sync.dma_start(mk_sb[:], mk_d[:]).then_inc(s_im, 16)
                for b in range(4):
                    sl = slice(8 * b, 8 * b + 8)
                    sync.wait_ge(s_cpa, 4 * (b + 1))
                    sync.wait_ge(s_cpv, 4 * (b + 1))
                    sync.dma_start(out_d[:, sl, :], o_sb[:, sl, :]).then_inc(
                        s_o, 16
                    )
                sync.wait_ge(s_cpa, 18)
                sync.wait_ge(s_cpv, 18)
                sync.dma_start(out_d[:, 32:36, :], o_sb[:, 32:36, :]).then_inc(
                    s_o, 16
                )
                sync.wait_ge(s_cpa, 19)
                sync.wait_ge(s_cpv, 19)
                sync.dma_start(out_d[:, 36:38, :], o_sb[:, 36:38, :]).then_inc(
                    s_o, 16
                )
                sync.wait_ge(s_o, 128)

            @block.tensor
            def _(pe):
                # warmup on the zeroed wz tile while input DMAs land; also
                # ramps the PE p-state so phase 1 runs near full clock
                pe.wait_ge(s_wz, 1)
                for w_ in range(27):
                    nc.tensor.matmul(
                        ps[7][0:128, 0:128],
                        wz[:, 0:128],
                        wz[:, 0:128],
                        start=(w_ == 0),
                        stop=(w_ == 26),
                    )
                pe.wait_ge(s_izw[0], 16)
                # phase 1: chunk blk accumulates in bank blk.
                # Order: ip0 across all chunks (only q piece 0 needed, so
                # compute starts while the rest of q is in flight), then
                # ip1..ip4 for chunks 0/1 (their f feeds phase-2 early),
                # then chunks 2..5 chunk-major for staggered exp/f.
                def p1_mm(blk, ip, j, start, stop):
                    return nc.tensor.matmul(
                        ps[blk][0:128, 0:PQ],
                        zw_pair(128 * (blk + ip) + j),
                        q_pair(ip, j),
                        start=start,
                        stop=stop,
                        perf_mode=PM.DoubleRow,
                    )

                pe.wait_ge(s_qs[0], 16)
                for blk in range(NB):
                    for j in range(KW):
                        p1_mm(blk, 0, j, j == 0, False)
                for ip in range(1, NIP):
                    pe.wait_ge(s_qs[ip], 16)
                    for blk in (0, 1):
                        for j in range(KW):
                            mm = p1_mm(
                                blk, ip, j, False, ip == NIP - 1 and j == KW - 1
                            )
                        if ip == NIP - 1:
                            mm.then_inc(s_p, 1)  # chunks 0,1 -> s_p 1,2
                for blk in range(2, NB):
                    for ip in range(1, NIP):
                        for j in range(KW):
                            mm = p1_mm(
                                blk, ip, j, False, ip == NIP - 1 and j == KW - 1
                            )
                    mm.then_inc(s_p, 1)  # chunks 2..5 -> s_p 3..6

                # phase 2: tap g -> bank PERM[g%8], 3 DoubleRow per tap
                def p2_mm(g, m, start, stop):
                    i_, j_ = g // KW, g % KW
                    di = 4 * (i_ % 2) + j_
                    a0 = 2 * m + i_ // 2
                    return nc.tensor.matmul(
                        ps[PERM[g % 8]][0:128, 0:PQ],
                        zt_sb[:, di, a0 : a0 + 2, :],
                        f_sb[:, 2 * m : 2 * m + 2, :],
                        start=start,
                        stop=stop,
                        perf_mode=PM.DoubleRow,
                    )

                # batch 0 m-major: start as soon as f0/f1 exist, hide the
                # phase-1 tail (exp5/f5) behind taps' m=0/m=1 passes
                pe.wait_ge(s_izt[0], 16)
                pe.wait_ge(s_f, 2)
                for g in range(8):
                    if g >= 4:
                        pe.wait_ge(s_exp, g - 1)  # bank g-2 freed by exp
                    p2_mm(g, 0, True, False)
                pe.wait_ge(s_f, 4)
                for g in range(8):
                    p2_mm(g, 1, False, False)
                pe.wait_ge(s_f, NB)
                pe.wait_ge(s_izt[1], 16)
                for g in range(8):
                    p2_mm(g, 2, False, True).then_inc(s_p, 1)
                # batches 1..4 g-major
                for g in range(8, NIJ):
                    gp = g - 8
                    if g % 2 == 0:
                        pe.wait_ge(s_cpa, gp // 2 + 1)
                    else:
                        pe.wait_ge(s_cpv, gp // 2 + 1)
                    if g == 33:
                        # bank 7 was reused by the den chain: wait its copy
                        pe.wait_ge(s_den, 1)
                    p2_mm(g, 0, True, False)
                    p2_mm(g, 1, False, False)
                    p2_mm(g, 2, False, True).then_inc(s_p, 1)
                    if g == DEN_AT:
                        # denominator: ones.T @ f -> [1, 160] in bank 7
                        # (tap 25's copy freed it: s_cpv >= 13)
                        pe.wait_ge(s_wz, 2)
                        pe.wait_ge(s_cpv, 13)
                        for ci in range(NB):
                            mm = nc.tensor.matmul(
                                ps[7][0:1, 0:PQ],
                                ones_sb[0:128, 0:1],
                                f_sb[:, ci, :],
                                start=(ci == 0),
                                stop=(ci == NB - 1),
                            )
                        mm.then_inc(s_p, 1)  # s_p = 36

            @block.scalar
            def _(act):
                # input DMAs on the Activation HW-DGE queue, consumption order
                act.dma_start(q_sb[:, 0:1], q_d[:, 0:1]).then_inc(s_qs[0], 16)
                act.dma_start(q_sb[:, 2:3], q_d[:, 2:3]).then_inc(s_qs[2], 16)
                act.dma_start(q_sb[:, 4:5], q_d[:, 4:5]).then_inc(s_qs[4], 16)
                act.dma_start(zt_sb[:, :, 0:5, :], zt_d[:, :, 0:5, :]).then_inc(
                    s_izt[0], 16
                )
                act.dma_start(zt_sb[:, :, 5:NA, :], zt_d[:, :, 5:NA, :]).then_inc(
                    s_izt[1], 16
                )
                for ci in range(NB):
                    act.wait_ge(s_p, ci + 1)
                    nc.scalar.activation(
                        e_sb[:, ci, :], ps[ci][0:128, 0:PQ], AF.Exp, scale=SCALE
                    ).then_inc(s_exp, 1)
                for g in range(0, NIJ, 2):
                    act.wait_ge(s_p, sp_tap(g))
                    nc.scalar.activation(
                        o_sb[:, g, :], ps[PERM[g % 8]][0:128, 0:PQ], AF.Copy
                    ).then_inc(s_cpa, 1)
                    if g == DEN_AT:
                        act.wait_ge(s_p, 36)
                        nc.scalar.activation(
                            den_sb[0:1, 0:PQ], ps[7][0:1, 0:PQ], AF.Copy
                        ).then_inc(s_den, 1)
                        act.wait_ge(s_den, 1)
                        act.dma_start(
                            den_d[:, :], den_sb[0:1, 0:PQ]
                        ).then_inc(s_o, 16)
                # tail outputs on this queue so they overlap the SP groups
                act.wait_ge(s_cpa, 20)
                act.wait_ge(s_cpv, 20)
                act.dma_start(out_d[:, 38:NIJ, :], o_sb[:, 38:NIJ, :]).then_inc(
                    s_o, 16
                )

            @block.vector
            def _(dve):
                nc.vector.memset(wz[:], 0.0).then_inc(s_wz, 1)
                nc.vector.memset(ones_sb[:], 1.0).then_inc(s_wz, 1)
                dve.wait_ge(s_im, 16)  # mask resident
                for ci in range(NB):
                    dve.wait_ge(s_exp, ci + 1)
                    nc.vector.tensor_scalar(
                        f_sb[:, ci, :],
                        e_sb[:, ci, :],
                        -1.0,
                        mk_sb[:, ci : ci + 1],
                        ALU.add,
                        ALU.mult,
                    ).then_inc(s_f, 1)
                for g in range(1, NIJ, 2):
                    dve.wait_ge(s_p, sp_tap(g))
                    nc.vector.tensor_copy(
                        o_sb[:, g, :], ps[PERM[g % 8]][0:128, 0:PQ]
                    ).then_inc(s_cpv, 1)

    return nc


def _host_prep(z1_hat, z2):
    z1 = np.asarray(z1_hat, dtype=np.float32)[0]   # [128, 100, 64]
    z2a = np.asarray(z2, dtype=np.float32)[0]

    # q taps interleaved as DoubleRow pairs (i, i+1):
    # qp[c, ip, j, t, pq] = q4[c, 4*(2*ip+t)+j, pq]
    q = z1.reshape(KC, NH, KH, NW, KW).transpose(1, 3, 0, 2, 4).reshape(PQ, D)
    q4 = q.reshape(PQ, KC, KH, KW).transpose(1, 2, 3, 0)   # [128, 10, 4, 160]
    qp = np.ascontiguousarray(
        q4.reshape(KC, NIP, 2, KW, PQ)
        .transpose(0, 1, 3, 2, 4)
        .reshape(KC, NIP, KW * 2 * PQ)
        .astype(F8)
    )

    z_pad = np.zeros((KC, 112, W), dtype=np.float32)
    z_pad[:, :H] = z2a

    in_maps = []
    p = np.arange(128)
    for core in range(NCORES):
        h0 = HPC * core
        slab = z_pad[:, h0 : h0 + 24, :].reshape(KC, NPOS)  # [128, 1536] f32
        zw = np.ascontiguousarray(slab.astype(F8))

        # zt[p, di, a, c] = slabT[128*a + DELTAS[di] + p, c]
        slabT = slab.T                                       # [1536, 128]
        zt = np.zeros((128, 8, NA, KC), dtype=F8)
        for di, d in enumerate(DELTAS):
            for a in range(NA):
                zt[:, di, a, :] = slabT[128 * a + d : 128 * a + d + 128]

        # masks, natural position order: x = 128*ci + p
        msk = np.zeros((128, NB), dtype=np.float32)
        for ci in range(NB):
            x = 128 * ci + p
            real = ((x % W) < WK) & ((h0 + x // W) < HK)
            msk[:, ci] = real
        in_maps.append(
            {
                "zw": zw,
                "qp": qp,
                "zt": np.ascontiguousarray(zt),
                "msk": msk,
            }
        )

    # colsum[(c,i,j)] = sum of kv rows over real patches, via integral image
    I = np.zeros((KC, H + 1, W + 1), dtype=np.float64)
    I[:, 1:, 1:] = z2a.astype(np.float64).cumsum(axis=1).cumsum(axis=2)
    colsum = np.zeros((KC, KH, KW), dtype=np.float64)
    for i in range(KH):
        for j in range(KW):
            colsum[:, i, j] = (
                I[:, i + HK, j + WK] - I[:, i, j + WK] - I[:, i + HK, j] + I[:, i, j]
            )
    return in_maps, colsum.reshape(KC, NIJ)


def kernel(z1_hat, z2):
    from concourse.bass_utils import run_bass_kernel_spmd

    in_maps, colsum = _host_prep(z1_hat, z2)
    if "nc" not in _CACHE:
        _CACHE["nc"] = _build_nc()
    nc = _CACHE["nc"]
    res = run_bass_kernel_spmd(nc, in_maps, list(range(NCORES)))
    num = colsum[:, :, None].astype(np.float64).copy()     # [128, 40, 1]
    num = np.broadcast_to(num, (KC, NIJ, PQ)).copy()
    den = np.full((PQ,), float(HK * WK), dtype=np.float64)
    for r in res.results:
        num += r["out"].astype(np.float64)
        den += r["den"].astype(np.float64)[0]
    out = num / den[None, None, :]
    # fold: [c, (i,j), q=(nh,nw)] -> [1, 128, 100, 64]
    arr = out.reshape(KC, KH, KW, NH, NW).transpose(0, 3, 1, 4, 2)
    return np.ascontiguousarray(arr.reshape(1, KC, H, W).astype(np.float32))


# revision 23
# speedup vs baseline: 1.0114x; 1.0114x over previous
"""Trainium2 Bass kernel for BottleneckAttention (patch attention).

q patches [160, 5120] from z1_hat (non-overlapping 10x4 unfold),
kv patches [5551, 5120] from z2 (overlapping unfold, Hk=91 x Wk=61),
scores = q @ kv.T / 5120, softmax over kv patches, out = attn @ kv,
folded back to [1, 128, 100, 64].

Sharding: 12 kv h-rows (768 flat positions) per core; every core computes
all 160 q columns; host combines with an all-gather softmax using the
centered form f = e - 1 (the exact colsum term is added in fp64 on host).

Per-core kernel (v14): every matmul is an fp8e4 DoubleRow matmul -- two
128-contraction k-tiles per instruction at ~78 ns (vs ~68 ns for one),
which beats the previous Winograd scheme outright and removes all the
vector-engine score combining.
  phase 1 computes scores TRANSPOSED [pos, q]: per 128-position chunk,
    taps pair over (i, i+1): the stationary pair is an overlapping
    [128, 2, 128] window AP (dim-1 stride 64) on the flat z slab; the
    moving pair is the host-interleaved q tap pair [128, 2, 160].
    Order: ip0 across all 6 chunks (only q piece 0 needed, so compute
    starts while the rest of q is still in flight), then ip1..4 for
    chunks 0/1 (their f feeds phase 2 early), then chunks 2..5.
    ScalarE applies exp straight from PSUM; VectorE forms f = (e-1)*mask
    in fp8.
  phase 2 computes out TRANSPOSED [(c,i,j), q]: per tap, 3 DoubleRow
    over position-chunk pairs: stationary = adjacent [128, 2, 128] slabs
    of the host-built zt (shifted z2T copies), moving = f chunk pairs.
    Batch 0 (taps 0-7) runs m-major so the phase-1 exp/f tail hides
    behind its m=0/m=1 passes. PSUM->SBUF copies alternate
    ScalarE/VectorE; fp16 output.
  denominator = ones-vector matmul chain after tap 28 in bank 6, shipped
    immediately on the Activation queue.
Input DMAs ride both HW-DGE queues (Activation: q0/q2/q4 + zt halves;
SP/Sync: zw, q1, q3, mask) in consumption order -- DMA completion has a
~2 us notify latency, so phase 1's start is input-bound; the 27 warmup
matmuls keep the PE p-state ramped until q piece 0 lands. Outputs ride
the SP queue in 8-tap groups as copies complete, with the last group
split off on the Activation queue.
"""

import sys

sys.path.insert(0, "/opt/trn_rl_repo")

import numpy as np
import ml_dtypes

import concourse.bass as bass
import concourse.mybir as mybir
import bass_rust

DT = mybir.dt
AF = mybir.ActivationFunctionType
ALU = mybir.AluOpType
PM = mybir.MatmulPerfMode

# problem geometry (hardcoded from the reference module)
KC, KH, KW = 128, 10, 4
H, W = 100, 64
NH, NW = H // KH, W // KW          # 10, 16
PQ = NH * NW                       # 160 q patches
D = KC * KH * KW                   # 5120
HK, WK = H - KH + 1, W - KW + 1    # 91, 61
NCORES = 8
HPC = 12                           # kv h-rows per core
NPOS = 24 * W                      # 1536 slab positions per core
NOWN = HPC * W                     # 768 owned positions per core
NB = NOWN // 128                   # 6 score/f chunks of 128 positions
NIP = KH // 2                      # 5 i-pairs
NIJ = KH * KW                      # 40 (i,j) output taps
NA = 10                            # zt 'a' slabs
DELTAS = (0, 1, 2, 3, 64, 65, 66, 67)
SCALE = 1.0 / D

F8 = ml_dtypes.float8_e4m3

_CACHE = {}

# phase-2 tap -> psum bank (taps 0,1 use the banks phase 1 never touches)
PERM = (3, 4, 5, 6, 7, 0, 1, 2)


def _build_nc():
    nc = bass.Bass()
    zw_d = nc.declare_dram_parameter("zw", [KC, NPOS], DT.float8e4, isOutput=False)
    q_d = nc.declare_dram_parameter(
        "qp", [KC, NIP, KW * 2 * PQ], DT.float8e4, isOutput=False
    )
    zt_d = nc.declare_dram_parameter("zt", [128, 8, NA, KC], DT.float8e4, isOutput=False)
    mk_d = nc.declare_dram_parameter("msk", [128, NB], DT.float32, isOutput=False)
    out_d = nc.declare_dram_parameter("out", [KC, NIJ, PQ], DT.float16, isOutput=True)
    den_d = nc.declare_dram_parameter("den", [1, PQ], DT.float32, isOutput=True)

    from contextlib import ExitStack

    ctx = ExitStack()
    with ctx:
        zw_sb = ctx.enter_context(nc.sbuf_tensor([KC, NPOS], DT.float8e4))
        q_sb = ctx.enter_context(nc.sbuf_tensor([KC, NIP, KW * 2 * PQ], DT.float8e4))
        zt_sb = ctx.enter_context(nc.sbuf_tensor([128, 8, NA, KC], DT.float8e4))
        mk_sb = ctx.enter_context(nc.sbuf_tensor([128, NB], DT.float32))
        e_sb = ctx.enter_context(nc.sbuf_tensor([128, NB, PQ], DT.float32))
        f_sb = ctx.enter_context(nc.sbuf_tensor([128, NB, PQ], DT.float8e4))
        o_sb = ctx.enter_context(nc.sbuf_tensor([128, NIJ, PQ], DT.float16))
        den_sb = ctx.enter_context(nc.sbuf_tensor([1, PQ], DT.float32))
        ones_sb = ctx.enter_context(nc.sbuf_tensor([128, 1], DT.float8e4))
        wz = ctx.enter_context(nc.sbuf_tensor([128, 128], DT.float8e4))

        ps = [
            ctx.enter_context(nc.psum_tensor(f"ps{i}", [128, 512], DT.float32))
            for i in range(8)
        ]

        s_wz = ctx.enter_context(nc.semaphore("s_wz"))
        s_izw = [ctx.enter_context(nc.semaphore(f"s_izw{i}")) for i in range(2)]
        s_qs = [ctx.enter_context(nc.semaphore(f"s_q{i}")) for i in range(NIP)]
        s_im = ctx.enter_context(nc.semaphore("s_im"))
        s_izt = [ctx.enter_context(nc.semaphore(f"s_izt{i}")) for i in range(2)]
        s_p = ctx.enter_context(nc.semaphore("s_p"))
        s_exp = ctx.enter_context(nc.semaphore("s_exp"))
        s_f = ctx.enter_context(nc.semaphore("s_f"))
        s_cpa = ctx.enter_context(nc.semaphore("s_cpa"))
        s_cpv = ctx.enter_context(nc.semaphore("s_cpv"))
        s_den = ctx.enter_context(nc.semaphore("s_den"))
        s_o = ctx.enter_context(nc.semaphore("s_o"))

        # s_p: phase-1 chunks 1..6; taps 0..28 -> 7..35; den -> 36;
        #      taps 29..39 -> 37..47
        DEN_AT = 28

        def sp_tap(g):
            return 7 + g if g <= DEN_AT else 8 + g

        def q_pair(ip, j):
            # contiguous tap pair [128, 2, 160] inside the flat q piece
            c = q_sb[:, ip, 2 * PQ * j : 2 * PQ * (j + 1)].copy()
            c.ap = bass_rust.VecI64Pair([[NIP * 2 * PQ * KW, 128], [PQ, 2], [1, PQ]])
            return c

        def q_pair4(ip, j):
            # fused moving [128, 2(t), 2(sel: chunk-odd@ip-1 | chunk-even@ip), 160]
            c = q_sb[:, ip - 1, 2 * PQ * j : 2 * PQ * j + PQ].copy()
            c.ap = bass_rust.VecI64Pair(
                [[NIP * 2 * PQ * KW, 128], [PQ, 2], [2 * PQ * KW, 2], [1, PQ]]
            )
            return c

        def zw_pair(off):
            # overlapping [128, 2, 128] stationary: windows at off, off+64
            c = zw_sb[:, off : off + 128].copy()
            c.ap = bass_rust.VecI64Pair([[NPOS, 128], [64, 2], [1, 128]])
            return c

        with nc.Block() as block:

            @block.sync
            def _(sync):
                # head inputs interleaved with the Activation queue: this
                # queue carries zw (split), q pieces 1/3 and the mask
                sync.dma_start(zw_sb[:], zw_d[:]).then_inc(s_izw[0], 16)
                sync.dma_start(q_sb[:, 1:2], q_d[:, 1:2]).then_inc(s_qs[1], 16)
                sync.dma_start(q_sb[:, 3:4], q_d[:, 3:4]).then_inc(s_qs[3], 16)
                sync.dma_start(mk_sb[:], mk_d[:]).then_inc(s_im, 16)
                for b in range(4):
                    sl = slice(8 * b, 8 * b + 8)
                    sync.wait_ge(s_cpa, 4 * (b + 1))
                    sync.wait_ge(s_cpv, 4 * (b + 1))
                    sync.dma_start(out_d[:, sl, :], o_sb[:, sl, :]).then_inc(
                        s_o, 16
                    )
                sync.wait_ge(s_cpa, 18)
                sync.wait_ge(s_cpv, 18)
                sync.dma_start(out_d[:, 32:36, :], o_sb[:, 32:36, :]).then_inc(
                    s_o, 16
                )
                sync.wait_ge(s_cpa, 19)
                sync.wait_ge(s_cpv, 19)
                sync.dma_start(out_d[:, 36:38, :], o_sb[:, 36:38, :]).then_inc(
                    s_o, 16
                )
                sync.wait_ge(s_o, 128)

            @block.tensor
            def _(pe):
                # warmup on the zeroed wz tile while input DMAs land; also
                # ramps the PE p-state so phase 1 runs near full clock
                pe.wait_ge(s_wz, 1)
                for w_ in range(27):
                    nc.tensor.matmul(
                        ps[7][0:128, 0:128],
                        wz[:, 0:128],
                        wz[:, 0:128],
                        start=(w_ == 0),
                        stop=(w_ == 26),
                    )
                pe.wait_ge(s_izw[0], 16)
                # phase 1, chunk-pair fused: pair P = (2p, 2p+1) shares bank
                # p. Window off = 128*(2p+ip)+j serves chunk 2p at tap-row
                # ip AND chunk 2p+1 at ip-1, so the middle stages are 320-col
                # fused DoubleRow (4-D moving AP: sel picks the q piece).
                # Bank p cols 160:320 = chunk 2p, cols 0:160 = chunk 2p+1.
                pe.wait_ge(s_qs[0], 16)
                for p in range(3):
                    for j in range(KW):
                        nc.tensor.matmul(
                            ps[p][0:128, PQ : 2 * PQ],
                            zw_pair(128 * (2 * p) + j),
                            q_pair(0, j),
                            start=(j == 0),
                            stop=False,
                            perf_mode=PM.DoubleRow,
                        )
                for p in range(3):
                    for ip in range(1, NIP):
                        if p == 0:
                            pe.wait_ge(s_qs[ip], 16)
                        for j in range(KW):
                            mm = nc.tensor.matmul(
                                ps[p][0:128, 0 : 2 * PQ],
                                zw_pair(128 * (2 * p + ip) + j),
                                q_pair4(ip, j),
                                start=False,
                                stop=False,
                                perf_mode=PM.DoubleRow,
                            )
                        if ip == NIP - 1:
                            mm.then_inc(s_p, 1)  # chunk 2p -> s_p 2p+1
                    for j in range(KW):
                        mm = nc.tensor.matmul(
                            ps[p][0:128, 0:PQ],
                            zw_pair(128 * (2 * p + 1 + 4) + j),
                            q_pair(4, j),
                            start=False,
                            stop=(j == KW - 1),
                            perf_mode=PM.DoubleRow,
                        )
                    mm.then_inc(s_p, 1)  # chunk 2p+1 -> s_p 2p+2

                # phase 2: tap g -> bank PERM[g%8], 3 DoubleRow per tap
                def p2_mm(g, m, start, stop):
                    i_, j_ = g // KW, g % KW
                    di = 4 * (i_ % 2) + j_
                    a0 = 2 * m + i_ // 2
                    return nc.tensor.matmul(
                        ps[PERM[g % 8]][0:128, 0:PQ],
                        zt_sb[:, di, a0 : a0 + 2, :],
                        f_sb[:, 2 * m : 2 * m + 2, :],
                        start=start,
                        stop=stop,
                        perf_mode=PM.DoubleRow,
                    )

                # batch 0 m-major: start as soon as f0/f1 exist, hide the
                # phase-1 tail (exp5/f5) behind taps' m=0/m=1 passes
                pe.wait_ge(s_izt[0], 16)
                pe.wait_ge(s_f, 2)
                B0GATE = {5: 2, 6: 4, 7: 6}
                for g in range(8):
                    if g in B0GATE:
                        pe.wait_ge(s_exp, B0GATE[g])  # score bank freed
                    p2_mm(g, 0, True, False)
                pe.wait_ge(s_f, 4)
                for g in range(8):
                    p2_mm(g, 1, False, False)
                pe.wait_ge(s_f, NB)
                pe.wait_ge(s_izt[1], 16)
                for g in range(8):
                    p2_mm(g, 2, False, True).then_inc(s_p, 1)
                # batches 1..4 g-major
                for g in range(8, NIJ):
                    gp = g - 8
                    if g % 2 == 0:
                        pe.wait_ge(s_cpa, gp // 2 + 1)
                    else:
                        pe.wait_ge(s_cpv, gp // 2 + 1)
                    if g == 33:
                        # bank 7 was reused by the den chain: wait its copy
                        pe.wait_ge(s_den, 1)
                    p2_mm(g, 0, True, False)
                    p2_mm(g, 1, False, False)
                    p2_mm(g, 2, False, True).then_inc(s_p, 1)
                    if g == DEN_AT:
                        # denominator: ones.T @ f -> [1, 160] in bank 7
                        # (tap 25's copy freed it: s_cpv >= 13)
                        pe.wait_ge(s_wz, 2)
                        pe.wait_ge(s_cpv, 13)
                        for ci in range(NB):
                            mm = nc.tensor.matmul(
                                ps[4][0:1, 0:PQ],
                                ones_sb[0:128, 0:1],
                                f_sb[:, ci, :],
                                start=(ci == 0),
                                stop=(ci == NB - 1),
                            )
                        mm.then_inc(s_p, 1)  # s_p = 36

            @block.scalar
            def _(act):
                # input DMAs on the Activation HW-DGE queue, consumption order
                act.dma_start(q_sb[:, 0:1], q_d[:, 0:1]).then_inc(s_qs[0], 16)
                act.dma_start(q_sb[:, 2:3], q_d[:, 2:3]).then_inc(s_qs[2], 16)
                act.dma_start(q_sb[:, 4:5], q_d[:, 4:5]).then_inc(s_qs[4], 16)
                act.dma_start(zt_sb[:, :, 0:5, :], zt_d[:, :, 0:5, :]).then_inc(
                    s_izt[0], 16
                )
                act.dma_start(zt_sb[:, :, 5:NA, :], zt_d[:, :, 5:NA, :]).then_inc(
                    s_izt[1], 16
                )
                for ci in range(NB):
                    # wait the whole bank's group closed (odd chunk's stop)
                    act.wait_ge(s_p, (ci | 1) + 1)
                    c0 = PQ if ci % 2 == 0 else 0
                    nc.scalar.activation(
                        e_sb[:, ci, :],
                        ps[ci // 2][0:128, c0 : c0 + PQ],
                        AF.Exp,
                        scale=SCALE,
                    ).then_inc(s_exp, 1)
                for g in range(0, NIJ, 2):
                    act.wait_ge(s_p, sp_tap(g))
                    nc.scalar.activation(
                        o_sb[:, g, :], ps[PERM[g % 8]][0:128, 0:PQ], AF.Copy
                    ).then_inc(s_cpa, 1)
                    if g == DEN_AT:
                        act.wait_ge(s_p, 36)
                        nc.scalar.activation(
                            den_sb[0:1, 0:PQ], ps[4][0:1, 0:PQ], AF.Copy
                        ).then_inc(s_den, 1)
                        act.wait_ge(s_den, 1)
                        act.dma_start(
                            den_d[:, :], den_sb[0:1, 0:PQ]
                        ).then_inc(s_o, 16)
                # tail outputs on this queue so they overlap the SP groups
                act.wait_ge(s_cpa, 20)
                act.wait_ge(s_cpv, 20)
                act.dma_start(out_d[:, 38:NIJ, :], o_sb[:, 38:NIJ, :]).then_inc(
                    s_o, 16
                )

            @block.vector
            def _(dve):
                nc.vector.memset(wz[:], 0.0).then_inc(s_wz, 1)
                nc.vector.memset(ones_sb[:], 1.0).then_inc(s_wz, 1)
                dve.wait_ge(s_im, 16)  # mask resident
                for ci in range(NB):
                    dve.wait_ge(s_exp, ci + 1)
                    nc.vector.tensor_scalar(
                        f_sb[:, ci, :],
                        e_sb[:, ci, :],
                        -1.0,
                        mk_sb[:, ci : ci + 1],
                        ALU.add,
                        ALU.mult,
                    ).then_inc(s_f, 1)
                for g in range(1, NIJ, 2):
                    dve.wait_ge(s_p, sp_tap(g))
                    nc.vector.tensor_copy(
                        o_sb[:, g, :], ps[PERM[g % 8]][0:128, 0:PQ]
                    ).then_inc(s_cpv, 1)

    return nc


def _host_prep(z1_hat, z2):
    z1 = np.asarray(z1_hat, dtype=np.float32)[0]   # [128, 100, 64]
    z2a = np.asarray(z2, dtype=np.float32)[0]

    # q taps interleaved as DoubleRow pairs (i, i+1):
    # qp[c, ip, j, t, pq] = q4[c, 4*(2*ip+t)+j, pq]
    q = z1.reshape(KC, NH, KH, NW, KW).transpose(1, 3, 0, 2, 4).reshape(PQ, D)
    q4 = q.reshape(PQ, KC, KH, KW).transpose(1, 2, 3, 0)   # [128, 10, 4, 160]
    qp = np.ascontiguousarray(
        q4.reshape(KC, NIP, 2, KW, PQ)
        .transpose(0, 1, 3, 2, 4)
        .reshape(KC, NIP, KW * 2 * PQ)
        .astype(F8)
    )

    z_pad = np.zeros((KC, 112, W), dtype=np.float32)
    z_pad[:, :H] = z2a

    in_maps = []
    p = np.arange(128)
    for core in range(NCORES):
        h0 = HPC * core
        slab = z_pad[:, h0 : h0 + 24, :].reshape(KC, NPOS)  # [128, 1536] f32
        zw = np.ascontiguousarray(slab.astype(F8))

        # zt[p, di, a, c] = slabT[128*a + DELTAS[di] + p, c]
        slabT = slab.T                                       # [1536, 128]
        zt = np.zeros((128, 8, NA, KC), dtype=F8)
        for di, d in enumerate(DELTAS):
            for a in range(NA):
                zt[:, di, a, :] = slabT[128 * a + d : 128 * a + d + 128]

        # masks, natural position order: x = 128*ci + p
        msk = np.zeros((128, NB), dtype=np.float32)
        for ci in range(NB):
            x = 128 * ci + p
            real = ((x % W) < WK) & ((h0 + x // W) < HK)
            msk[:, ci] = real
        in_maps.append(
            {
                "zw": zw,
                "qp": qp,
                "zt": np.ascontiguousarray(zt),
                "msk": msk,
            }
        )

    # colsum[(c,i,j)] = sum of kv rows over real patches, via integral image
    I = np.zeros((KC, H + 1, W + 1), dtype=np.float64)
    I[:, 1:, 1:] = z2a.astype(np.float64).cumsum(axis=1).cumsum(axis=2)
    colsum = np.zeros((KC, KH, KW), dtype=np.float64)
    for i in range(KH):
        for j in range(KW):
            colsum[:, i, j] = (
                I[:, i + HK, j + WK] - I[:, i, j + WK] - I[:, i + HK, j] + I[:, i, j]
            )
    return in_maps, colsum.reshape(KC, NIJ)


def kernel(z1_hat, z2):
    from concourse.bass_utils import run_bass_kernel_spmd

    in_maps, colsum = _host_prep(z1_hat, z2)
    if "nc" not in _CACHE:
        _CACHE["nc"] = _build_nc()
    nc = _CACHE["nc"]
    res = run_bass_kernel_spmd(nc, in_maps, list(range(NCORES)))
    num = colsum[:, :, None].astype(np.float64).copy()     # [128, 40, 1]
    num = np.broadcast_to(num, (KC, NIJ, PQ)).copy()
    den = np.full((PQ,), float(HK * WK), dtype=np.float64)
    for r in res.results:
        num += r["out"].astype(np.float64)
        den += r["den"].astype(np.float64)[0]
    out = num / den[None, None, :]
    # fold: [c, (i,j), q=(nh,nw)] -> [1, 128, 100, 64]
    arr = out.reshape(KC, KH, KW, NH, NW).transpose(0, 3, 1, 4, 2)
    return np.ascontiguousarray(arr.reshape(1, KC, H, W).astype(np.float32))


# revision 29
# speedup vs baseline: 1.0394x; 1.0276x over previous
"""Trainium2 Bass kernel for BottleneckAttention (patch attention).

q patches [160, 5120] from z1_hat (non-overlapping 10x4 unfold),
kv patches [5551, 5120] from z2 (overlapping unfold, Hk=91 x Wk=61),
scores = q @ kv.T / 5120, softmax over kv patches, out = attn @ kv,
folded back to [1, 128, 100, 64].

Sharding: 12 kv h-rows (768 flat positions) per core; every core computes
all 160 q columns; host combines with an all-gather softmax using the
centered form f = e - 1 (the exact colsum term is added in fp64 on host).

Per-core kernel (v20): every matmul is an fp8e4 DoubleRow matmul (two
128-contraction k-tiles per ~78 ns instruction).
  phase 1 computes scores TRANSPOSED [pos, q], chunk-pair FUSED: pair
    P = (2p, 2p+1) shares psum bank p (cols 160:320 = even chunk,
    0:160 = odd). The window at 128*(2p+ip)+j serves chunk 2p at
    tap-row ip AND chunk 2p+1 at ip-1, so the middle stages are 320-col
    fused DoubleRow with a 4-D moving AP (sel picks the q piece); the
    stationary is an overlapping [128, 2, 128] window AP (stride 64) on
    the flat z slab. Solo head (ip0, even chunk) and tail (ip4, odd
    chunk) stages bracket each pair; pairs 0/1 run stage-interleaved to
    match the q-piece DMA arrival spacing, each pair's tail follows its
    last fused stage so its bank closes early (stop order 0..5).
    ScalarE applies exp straight from PSUM halves once the bank's group
    is closed; VectorE forms f = (e-1)*mask in fp8.
  phase 2 computes out TRANSPOSED [(c,i,j), q]: per tap, 3 DoubleRow
    over position-chunk pairs: stationary = adjacent [128, 2, 128] slabs
    of the host-built zt (shifted z2T copies), moving = f chunk pairs.
    Batch 0 (taps 0-7, banks PERM) runs m-major with taps 6/7 last so
    their banks' exp4/exp5 drain hides behind taps 0-5's m0+m1 work.
    PSUM->SBUF copies alternate ScalarE/VectorE; fp16 output.
  denominator = ones-vector matmul chain after tap 28 in bank 4, shipped
    immediately on the Activation queue.
Input DMAs ride both HW-DGE queues (Activation: q0/q2/q4 + zt halves;
SP/Sync: zw, q1, q3, mask) in consumption order -- DMA completion has a
~2 us notify latency, so phase 1's start is input-bound; the 29 warmup
matmuls keep the PE p-state ramped until q piece 0 lands. Outputs ride
the SP queue in 8-tap groups as copies complete, with the last tiles
split off on the Activation queue.
"""

import sys

sys.path.insert(0, "/opt/trn_rl_repo")

import numpy as np
import ml_dtypes

import concourse.bass as bass
import concourse.mybir as mybir
import bass_rust

DT = mybir.dt
AF = mybir.ActivationFunctionType
ALU = mybir.AluOpType
PM = mybir.MatmulPerfMode

# problem geometry (hardcoded from the reference module)
KC, KH, KW = 128, 10, 4
H, W = 100, 64
NH, NW = H // KH, W // KW          # 10, 16
PQ = NH * NW                       # 160 q patches
D = KC * KH * KW                   # 5120
HK, WK = H - KH + 1, W - KW + 1    # 91, 61
NCORES = 8
HPC = 12                           # kv h-rows per core
NPOS = 24 * W                      # 1536 slab positions per core
NOWN = HPC * W                     # 768 owned positions per core
NB = NOWN // 128                   # 6 score/f chunks of 128 positions
NIP = KH // 2                      # 5 i-pairs
NIJ = KH * KW                      # 40 (i,j) output taps
NA = 10                            # zt 'a' slabs
DELTAS = (0, 1, 2, 3, 64, 65, 66, 67)
SCALE = 1.0 / D

F8 = ml_dtypes.float8_e4m3

_CACHE = {}

# phase-2 tap -> psum bank (taps 0,1 use the banks phase 1 never touches)
PERM = (3, 4, 5, 6, 7, 0, 1, 2)


def _build_nc():
    nc = bass.Bass()
    zw_d = nc.declare_dram_parameter("zw", [KC, NPOS], DT.float8e4, isOutput=False)
    q_d = nc.declare_dram_parameter(
        "qp", [KC, NIP, KW * 2 * PQ], DT.float8e4, isOutput=False
    )
    zt_d = nc.declare_dram_parameter("zt", [128, 8, NA, KC], DT.float8e4, isOutput=False)
    mk_d = nc.declare_dram_parameter("msk", [128, NB], DT.float32, isOutput=False)
    out_d = nc.declare_dram_parameter("out", [KC, NIJ, PQ], DT.float16, isOutput=True)
    den_d = nc.declare_dram_parameter("den", [1, PQ], DT.float32, isOutput=True)

    from contextlib import ExitStack

    ctx = ExitStack()
    with ctx:
        zw_sb = ctx.enter_context(nc.sbuf_tensor([KC, NPOS], DT.float8e4))
        q_sb = ctx.enter_context(nc.sbuf_tensor([KC, NIP, KW * 2 * PQ], DT.float8e4))
        zt_sb = ctx.enter_context(nc.sbuf_tensor([128, 8, NA, KC], DT.float8e4))
        mk_sb = ctx.enter_context(nc.sbuf_tensor([128, NB], DT.float32))
        e_sb = ctx.enter_context(nc.sbuf_tensor([128, NB, PQ], DT.float32))
        f_sb = ctx.enter_context(nc.sbuf_tensor([128, NB, PQ], DT.float8e4))
        o_sb = ctx.enter_context(nc.sbuf_tensor([128, NIJ, PQ], DT.float16))
        den_sb = ctx.enter_context(nc.sbuf_tensor([1, PQ], DT.float32))
        ones_sb = ctx.enter_context(nc.sbuf_tensor([128, 1], DT.float8e4))
        wz = ctx.enter_context(nc.sbuf_tensor([128, 128], DT.float8e4))

        ps = [
            ctx.enter_context(nc.psum_tensor(f"ps{i}", [128, 512], DT.float32))
            for i in range(8)
        ]

        s_wz = ctx.enter_context(nc.semaphore("s_wz"))
        s_izw = [ctx.enter_context(nc.semaphore(f"s_izw{i}")) for i in range(2)]
        s_qs = [ctx.enter_context(nc.semaphore(f"s_q{i}")) for i in range(NIP)]
        s_im = ctx.enter_context(nc.semaphore("s_im"))
        s_izt = [ctx.enter_context(nc.semaphore(f"s_izt{i}")) for i in range(2)]
        s_p = ctx.enter_context(nc.semaphore("s_p"))
        s_exp = ctx.enter_context(nc.semaphore("s_exp"))
        s_f = ctx.enter_context(nc.semaphore("s_f"))
        s_cpa = ctx.enter_context(nc.semaphore("s_cpa"))
        s_cpv = ctx.enter_context(nc.semaphore("s_cpv"))
        s_den = ctx.enter_context(nc.semaphore("s_den"))
        s_o = ctx.enter_context(nc.semaphore("s_o"))

        # s_p: phase-1 chunks 1..6; taps 0..28 -> 7..35; den -> 36;
        #      taps 29..39 -> 37..47
        DEN_AT = 28

        def sp_tap(g):
            return 7 + g if g <= DEN_AT else 8 + g

        def q_pair(ip, j):
            # contiguous tap pair [128, 2, 160] inside the flat q piece
            c = q_sb[:, ip, 2 * PQ * j : 2 * PQ * (j + 1)].copy()
            c.ap = bass_rust.VecI64Pair([[NIP * 2 * PQ * KW, 128], [PQ, 2], [1, PQ]])
            return c

        def q_pair4(ip, j):
            # fused moving [128, 2(t), 2(sel: chunk-odd@ip-1 | chunk-even@ip), 160]
            c = q_sb[:, ip - 1, 2 * PQ * j : 2 * PQ * j + PQ].copy()
            c.ap = bass_rust.VecI64Pair(
                [[NIP * 2 * PQ * KW, 128], [PQ, 2], [2 * PQ * KW, 2], [1, PQ]]
            )
            return c

        def zw_pair(off):
            # overlapping [128, 2, 128] stationary: windows at off, off+64
            c = zw_sb[:, off : off + 128].copy()
            c.ap = bass_rust.VecI64Pair([[NPOS, 128], [64, 2], [1, 128]])
            return c

        with nc.Block() as block:

            @block.sync
            def _(sync):
                # head inputs interleaved with the Activation queue: this
                # queue carries zw (split), q pieces 1/3 and the mask
                sync.dma_start(zw_sb[:], zw_d[:]).then_inc(s_izw[0], 16)
                sync.dma_start(q_sb[:, 1:2], q_d[:, 1:2]).then_inc(s_qs[1], 16)
                sync.dma_start(q_sb[:, 3:4], q_d[:, 3:4]).then_inc(s_qs[3], 16)
                sync.dma_start(mk_sb[:], mk_d[:]).then_inc(s_im, 16)
                for b in range(4):
                    sl = slice(8 * b, 8 * b + 8)
                    sync.wait_ge(s_cpa, 4 * (b + 1))
                    sync.wait_ge(s_cpv, 4 * (b + 1))
                    sync.dma_start(out_d[:, sl, :], o_sb[:, sl, :]).then_inc(
                        s_o, 16
                    )
                sync.wait_ge(s_cpa, 18)
                sync.wait_ge(s_cpv, 18)
                sync.dma_start(out_d[:, 32:36, :], o_sb[:, 32:36, :]).then_inc(
                    s_o, 16
                )
                sync.wait_ge(s_cpa, 19)
                sync.wait_ge(s_cpv, 19)
                sync.dma_start(out_d[:, 36:38, :], o_sb[:, 36:38, :]).then_inc(
                    s_o, 16
                )
                sync.wait_ge(s_o, 128)

            @block.tensor
            def _(pe):
                # warmup on the zeroed wz tile while input DMAs land; also
                # ramps the PE p-state so phase 1 runs near full clock
                pe.wait_ge(s_wz, 1)
                for w_ in range(29):
                    nc.tensor.matmul(
                        ps[7][0:128, 0:128],
                        wz[:, 0:128],
                        wz[:, 0:128],
                        start=(w_ == 0),
                        stop=(w_ == 28),
                    )
                pe.wait_ge(s_izw[0], 16)
                # phase 1, chunk-pair fused: pair P = (2p, 2p+1) shares bank
                # p. Window off = 128*(2p+ip)+j serves chunk 2p at tap-row
                # ip AND chunk 2p+1 at ip-1, so the middle stages are 320-col
                # fused DoubleRow (4-D moving AP: sel picks the q piece).
                # Bank p cols 160:320 = chunk 2p, cols 0:160 = chunk 2p+1.
                pe.wait_ge(s_qs[0], 16)
                for p in range(3):
                    for j in range(KW):
                        nc.tensor.matmul(
                            ps[p][0:128, PQ : 2 * PQ],
                            zw_pair(128 * (2 * p) + j),
                            q_pair(0, j),
                            start=(j == 0),
                            stop=False,
                            perf_mode=PM.DoubleRow,
                        )
                def p1_fused(p, ip, inc):
                    for j in range(KW):
                        mm = nc.tensor.matmul(
                            ps[p][0:128, 0 : 2 * PQ],
                            zw_pair(128 * (2 * p + ip) + j),
                            q_pair4(ip, j),
                            start=False,
                            stop=False,
                            perf_mode=PM.DoubleRow,
                        )
                    if inc:
                        mm.then_inc(s_p, 1)

                def p1_tail(p, inc=True):
                    for j in range(KW):
                        mm = nc.tensor.matmul(
                            ps[p][0:128, 0:PQ],
                            zw_pair(128 * (2 * p + 1 + 4) + j),
                            q_pair(4, j),
                            start=False,
                            stop=(j == KW - 1),
                            perf_mode=PM.DoubleRow,
                        )
                    if inc:
                        mm.then_inc(s_p, 1)

                # pairs 0,1 stage-interleaved (stage work ~1.1us matches the
                # q piece arrival spacing); pair 2 runs free afterwards.
                # Chunk stop order: 0, 2, 1, 3, 4, 5 -> s_p 1..6
                for ip in range(1, NIP - 1):
                    pe.wait_ge(s_qs[ip], 16)
                    p1_fused(0, ip, False)
                    p1_fused(1, ip, False)
                pe.wait_ge(s_qs[NIP - 1], 16)
                p1_fused(0, NIP - 1, True)            # chunk 0 -> s_p 1
                p1_tail(0)                            # chunk 1 -> s_p 2
                p1_fused(1, NIP - 1, True)            # chunk 2 -> s_p 3
                p1_tail(1)                            # chunk 3 -> s_p 4
                for ip in range(1, NIP):
                    p1_fused(2, ip, ip == NIP - 1)   # chunk 4 -> s_p 5
                p1_tail(2)                            # chunk 5 -> s_p 6

                # phase 2: tap g -> bank PERM[g%8], 3 DoubleRow per tap
                def p2_mm(g, m, start, stop):
                    i_, j_ = g // KW, g % KW
                    di = 4 * (i_ % 2) + j_
                    a0 = 2 * m + i_ // 2
                    return nc.tensor.matmul(
                        ps[PERM[g % 8]][0:128, 0:PQ],
                        zt_sb[:, di, a0 : a0 + 2, :],
                        f_sb[:, 2 * m : 2 * m + 2, :],
                        start=start,
                        stop=stop,
                        perf_mode=PM.DoubleRow,
                    )

                # batch 0 m-major: start as soon as f0/f1 exist, hide the
                # phase-1 tail (exp5/f5) behind taps' m=0/m=1 passes
                pe.wait_ge(s_izt[0], 16)
                pe.wait_ge(s_f, 2)
                for g in range(6):
                    if g == 5:
                        pe.wait_ge(s_exp, 2)  # bank 0 freed by exps 0,1
                    p2_mm(g, 0, True, False)
                pe.wait_ge(s_f, 4)
                for g in range(6):
                    p2_mm(g, 1, False, False)
                # taps 6,7 last: their banks 1,2 close at the phase-1 tail,
                # so the m0/m1 work above absorbs the exp4/exp5 drain
                pe.wait_ge(s_exp, 4)
                p2_mm(6, 0, True, False)
                pe.wait_ge(s_exp, NB)
                p2_mm(7, 0, True, False)
                p2_mm(6, 1, False, False)
                p2_mm(7, 1, False, False)
                pe.wait_ge(s_f, NB)
                pe.wait_ge(s_izt[1], 16)
                for g in range(8):
                    p2_mm(g, 2, False, True).then_inc(s_p, 1)
                # batches 1..4 g-major
                for g in range(8, NIJ):
                    gp = g - 8
                    if g % 2 == 0:
                        pe.wait_ge(s_cpa, gp // 2 + 1)
                    else:
                        pe.wait_ge(s_cpv, gp // 2 + 1)
                    if g == 33:
                        # bank 7 was reused by the den chain: wait its copy
                        pe.wait_ge(s_den, 1)
                    p2_mm(g, 0, True, False)
                    p2_mm(g, 1, False, False)
                    p2_mm(g, 2, False, True).then_inc(s_p, 1)
                    if g == DEN_AT:
                        # denominator: ones.T @ f -> [1, 160] in bank 7
                        # (tap 25's copy freed it: s_cpv >= 13)
                        pe.wait_ge(s_wz, 2)
                        pe.wait_ge(s_cpv, 13)
                        for ci in range(NB):
                            mm = nc.tensor.matmul(
                                ps[4][0:1, 0:PQ],
                                ones_sb[0:128, 0:1],
                                f_sb[:, ci, :],
                                start=(ci == 0),
                                stop=(ci == NB - 1),
                            )
                        mm.then_inc(s_p, 1)  # s_p = 36

            @block.scalar
            def _(act):
                # input DMAs on the Activation HW-DGE queue, consumption order
                act.dma_start(q_sb[:, 0:1], q_d[:, 0:1]).then_inc(s_qs[0], 16)
                act.dma_start(q_sb[:, 2:3], q_d[:, 2:3]).then_inc(s_qs[2], 16)
                act.dma_start(q_sb[:, 4:5], q_d[:, 4:5]).then_inc(s_qs[4], 16)
                act.dma_start(zt_sb[:, :, 0:5, :], zt_d[:, :, 0:5, :]).then_inc(
                    s_izt[0], 16
                )
                act.dma_start(zt_sb[:, :, 5:NA, :], zt_d[:, :, 5:NA, :]).then_inc(
                    s_izt[1], 16
                )
                EXPCLOSE = (2, 2, 4, 4, 6, 6)
                for ci in range(NB):
                    # wait the whole bank's group closed (odd chunk's stop)
                    act.wait_ge(s_p, EXPCLOSE[ci])
                    c0 = PQ if ci % 2 == 0 else 0
                    nc.scalar.activation(
                        e_sb[:, ci, :],
                        ps[ci // 2][0:128, c0 : c0 + PQ],
                        AF.Exp,
                        scale=SCALE,
                    ).then_inc(s_exp, 1)
                for g in range(0, NIJ, 2):
                    act.wait_ge(s_p, sp_tap(g))
                    nc.scalar.activation(
                        o_sb[:, g, :], ps[PERM[g % 8]][0:128, 0:PQ], AF.Copy
                    ).then_inc(s_cpa, 1)
                    if g == DEN_AT:
                        act.wait_ge(s_p, 36)
                        nc.scalar.activation(
                            den_sb[0:1, 0:PQ], ps[4][0:1, 0:PQ], AF.Copy
                        ).then_inc(s_den, 1)
                        act.wait_ge(s_den, 1)
                        act.dma_start(
                            den_d[:, :], den_sb[0:1, 0:PQ]
                        ).then_inc(s_o, 16)
                # tail outputs on this queue so they overlap the SP groups
                act.wait_ge(s_cpa, 20)
                act.wait_ge(s_cpv, 20)
                act.dma_start(out_d[:, 38:NIJ, :], o_sb[:, 38:NIJ, :]).then_inc(
                    s_o, 16
                )

            @block.vector
            def _(dve):
                nc.vector.memset(wz[:], 0.0).then_inc(s_wz, 1)
                nc.vector.memset(ones_sb[:], 1.0).then_inc(s_wz, 1)
                dve.wait_ge(s_im, 16)  # mask resident
                for ci in range(NB):
                    dve.wait_ge(s_exp, ci + 1)
                    nc.vector.tensor_scalar(
                        f_sb[:, ci, :],
                        e_sb[:, ci, :],
                        -1.0,
                        mk_sb[:, ci : ci + 1],
                        ALU.add,
                        ALU.mult,
                    ).then_inc(s_f, 1)
                for g in range(1, NIJ, 2):
                    dve.wait_ge(s_p, sp_tap(g))
                    nc.vector.tensor_copy(
                        o_sb[:, g, :], ps[PERM[g % 8]][0:128, 0:PQ]
                    ).then_inc(s_cpv, 1)

    return nc


def _host_prep(z1_hat, z2):
    z1 = np.asarray(z1_hat, dtype=np.float32)[0]   # [128, 100, 64]
    z2a = np.asarray(z2, dtype=np.float32)[0]

    # q taps interleaved as DoubleRow pairs (i, i+1):
    # qp[c, ip, j, t, pq] = q4[c, 4*(2*ip+t)+j, pq]
    q = z1.reshape(KC, NH, KH, NW, KW).transpose(1, 3, 0, 2, 4).reshape(PQ, D)
    q4 = q.reshape(PQ, KC, KH, KW).transpose(1, 2, 3, 0)   # [128, 10, 4, 160]
    qp = np.ascontiguousarray(
        q4.reshape(KC, NIP, 2, KW, PQ)
        .transpose(0, 1, 3, 2, 4)
        .reshape(KC, NIP, KW * 2 * PQ)
        .astype(F8)
    )

    z_pad = np.zeros((KC, 112, W), dtype=np.float32)
    z_pad[:, :H] = z2a

    in_maps = []
    p = np.arange(128)
    for core in range(NCORES):
        h0 = HPC * core
        slab = z_pad[:, h0 : h0 + 24, :].reshape(KC, NPOS)  # [128, 1536] f32
        zw = np.ascontiguousarray(slab.astype(F8))

        # zt[p, di, a, c] = slabT[128*a + DELTAS[di] + p, c]
        slabT = slab.T                                       # [1536, 128]
        zt = np.zeros((128, 8, NA, KC), dtype=F8)
        for di, d in enumerate(DELTAS):
            for a in range(NA):
                zt[:, di, a, :] = slabT[128 * a + d : 128 * a + d + 128]

        # masks, natural position order: x = 128*ci + p
        msk = np.zeros((128, NB), dtype=np.float32)
        for ci in range(NB):
            x = 128 * ci + p
            real = ((x % W) < WK) & ((h0 + x // W) < HK)
            msk[:, ci] = real
        in_maps.append(
            {
                "zw": zw,
                "qp": qp,
                "zt": np.ascontiguousarray(zt),
                "msk": msk,
            }
        )

    # colsum[(c,i,j)] = sum of kv rows over real patches, via integral image
    I = np.zeros((KC, H + 1, W + 1), dtype=np.float64)
    I[:, 1:, 1:] = z2a.astype(np.float64).cumsum(axis=1).cumsum(axis=2)
    colsum = np.zeros((KC, KH, KW), dtype=np.float64)
    for i in range(KH):
        for j in range(KW):
            colsum[:, i, j] = (
                I[:, i + HK, j + WK] - I[:, i, j + WK] - I[:, i + HK, j] + I[:, i, j]
            )
    return in_maps, colsum.reshape(KC, NIJ)


def kernel(z1_hat, z2):
    from concourse.bass_utils import run_bass_kernel_spmd

    in_maps, colsum = _host_prep(z1_hat, z2)
    if "nc" not in _CACHE:
        _CACHE["nc"] = _build_nc()
    nc = _CACHE["nc"]
    res = run_bass_kernel_spmd(nc, in_maps, list(range(NCORES)))
    num = colsum[:, :, None].astype(np.float64).copy()     # [128, 40, 1]
    num = np.broadcast_to(num, (KC, NIJ, PQ)).copy()
    den = np.full((PQ,), float(HK * WK), dtype=np.float64)
    for r in res.results:
        num += r["out"].astype(np.float64)
        den += r["den"].astype(np.float64)[0]
    out = num / den[None, None, :]
    # fold: [c, (i,j), q=(nh,nw)] -> [1, 128, 100, 64]
    arr = out.reshape(KC, KH, KW, NH, NW).transpose(0, 3, 1, 4, 2)
    return np.ascontiguousarray(arr.reshape(1, KC, H, W).astype(np.float32))
